# revision 16
# baseline (speedup 1.0000x reference)
"""Trainium2 Bass kernel for an input-feed GRU decoder with concat attention,
copy gate, and maxout readout.

Sharding: data-parallel over batch B=64 across 8 NeuronCores (8 batch rows per
core, params replicated). The T=32 recurrence is sequential; each core runs the
full scan for its batch slice with zero cross-core communication.

Layouts are feature-major: [feature-chunk-on-partitions, batch-on-free].
All matmuls run in bf16 (fp32 PSUM accumulate). Sigmoid is computed as
0.5 + 0.5*tanh(x/2) so the whole kernel uses one activation table set
(exp_and_others: tanh + exp + copy/identity).
"""

import sys

sys.path.insert(0, "/opt/trn_rl_repo")

from contextlib import ExitStack

import ml_dtypes
import numpy as np

import concourse.bass as bass
import concourse.tile as tile
from concourse import bacc, mybir
from concourse.bass_utils import run_bass_kernel_spmd

BF16 = mybir.dt.bfloat16
F32 = mybir.dt.float32
AF = mybir.ActivationFunctionType
ALU = mybir.AluOpType

# Problem shapes (hardcoded per contest rules).
T, B, S = 32, 64, 256
DEC = ENC = ATT = WV = 512
NCORES = 8
BL = B // NCORES  # 8 local batch
NMT = 12  # gate row tiles (3*DEC/128)
NKC = 4  # 512/128

_CACHE = {}


# --------------------------------------------------------------------------
# Device program
# --------------------------------------------------------------------------
def _build(nonzero_mask: bool, nonzero_bias: bool, b_copy_val: float,
           early_gates=True, psum_tgt=True, stt_gru=True, bcast=True, debug_t0=False):
    nc = bacc.Bacc("TRN2", target_bir_lowering=False, debug=False, num_devices=NCORES)

    def din(name, shape, dt=BF16):
        return nc.dram_tensor(name, shape, dt, kind="ExternalInput").ap()

    def dout(name, shape, dt=F32):
        return nc.dram_tensor(name, shape, dt, kind="ExternalOutput").ap()

    d_wh = din("wh", [128, 4 * 12 * 128])
    d_wx = din("wx", [128, 4 * 12 * 128])
    d_wq = din("wq", [128, 4 * 4 * 128])
    d_wr = din("wr", [128, 12 * 4 * 128])
    d_wp = din("wp", [128, 4 * 4 * 128])
    d_wcp = din("wcp", [128, 8])
    d_va = din("va", [128, 4])
    d_gxe = din("gxe", [128, 12 * T * BL])
    d_emb = din("emb", [128, 4 * T * BL])
    d_ce = din("ce", [128, 4 * BL * S])
    d_ct = din("ct", [128, BL * 2 * 4 * 128])
    d_h0 = din("h0", [128, 32])
    d_c0 = din("c0", [128, 32])
    d_ident = din("ident", [128, 128])
    d_onesc = din("onesc", [128, 1])
    d_onesr = din("onesr", [1, 128])
    if nonzero_mask:
        d_ma = din("ma", [128, 2 * BL], F32)
        d_mc = din("mc", [128, 2 * BL], F32)
    if nonzero_bias:
        d_bpre = din("bpre", [128, 4], F32)
        d_bhhn = din("bhhn", [128, 4], F32)
        d_brd = din("brd", [128, 4], F32)

    if debug_t0:
        d_dbg_psg = dout("dbg_psg", [128, 128])
        d_dbg_h = dout("dbg_h", [128, 32])
        d_dbg_tgt = dout("dbg_tgt", [128, 32])
        d_dbg_th = dout("dbg_th", [128, 4 * S])
    d_g = dout("g", [128, 2 * T * BL])
    d_co = dout("co", [128, 2 * T * BL], BF16)
    d_zs = dout("zs", [1, T * BL])
    d_hid = dout("hid", [128, 32])
    d_cctx = dout("cctx", [128, 32])

    with tile.TileContext(nc) as tc, ExitStack() as ctx:
        from contextlib import contextmanager

        @contextmanager
        def lowprio(bump=1_000_000):
            tc.cur_priority += bump
            try:
                yield
            finally:
                tc.cur_priority -= bump

        cpool = ctx.enter_context(tc.tile_pool(name="consts", bufs=1))
        hpool = ctx.enter_context(tc.tile_pool(name="hstate", bufs=2))
        cxpool = ctx.enter_context(tc.tile_pool(name="cstate", bufs=2))
        tmppool = ctx.enter_context(tc.tile_pool(name="tmp", bufs=3))
        thpool = ctx.enter_context(tc.tile_pool(name="th", bufs=3))
        grupool = ctx.enter_context(tc.tile_pool(name="gru", bufs=2))
        smpool = ctx.enter_context(tc.tile_pool(name="smx", bufs=2))

        ps_g = ctx.enter_context(tc.tile_pool(name="psg", bufs=2, space="PSUM"))
        ps_t = ctx.enter_context(tc.tile_pool(name="pst", bufs=1, space="PSUM"))
        ps_e = ctx.enter_context(tc.tile_pool(name="pse", bufs=1, space="PSUM"))
        ps_c = ctx.enter_context(tc.tile_pool(name="psc", bufs=1, space="PSUM"))
        ps_r = ctx.enter_context(tc.tile_pool(name="psr", bufs=1, space="PSUM"))
        ps_m = ctx.enter_context(tc.tile_pool(name="psm", bufs=1, space="PSUM"))
        ps_b = ctx.enter_context(tc.tile_pool(name="psb", bufs=1, space="PSUM"))

        # ---- constant loads ----
        def cload(dram, shape, dt=BF16):
            t_ = cpool.tile(shape, dt, tag=dram.tensor.name)
            nc.sync.dma_start(out=t_, in_=dram)
            return t_

        h0 = cload(d_h0, [128, 32])
        c0 = cload(d_c0, [128, 32])
        gxe = cload(d_gxe, [128, 12 * T * BL])
        wh = cload(d_wh, [128, 4 * 12 * 128])
        wx = cload(d_wx, [128, 4 * 12 * 128])
        ident = cload(d_ident, [128, 128])
        wp = cload(d_wp, [128, 4 * 4 * 128])
        ce = cload(d_ce, [128, 4 * BL * S])
        wq = cload(d_wq, [128, 4 * 4 * 128])
        va = cload(d_va, [128, 4])
        ct = cload(d_ct, [128, BL * 2 * 4 * 128])
        wr = cload(d_wr, [128, 12 * 4 * 128])
        wcp = cload(d_wcp, [128, 8])
        emb = cload(d_emb, [128, 4 * T * BL])
        onesc = cload(d_onesc, [128, 1])
        onesr = cload(d_onesr, [1, 128])
        ma = cload(d_ma, [128, 2 * BL], F32) if nonzero_mask else None
        mc = cload(d_mc, [128, 2 * BL], F32) if nonzero_mask else None
        bpre = cload(d_bpre, [128, 4], F32) if nonzero_bias else None
        bhhn = cload(d_bhhn, [128, 4], F32) if nonzero_bias else None
        brd = cload(d_brd, [128, 4], F32) if nonzero_bias else None

        ones_inv = cpool.tile([128, 1, BL], F32)
        nc.vector.memset(ones_inv[:], 1.0)
        pbuf = cpool.tile([128, BL * 4 * S], BF16)  # tanh arg precompute
        g_buf = cpool.tile([128, 2, T, BL], F32)
        co_buf = cpool.tile([128, 2, T, BL], BF16)
        zs_buf = cpool.tile([1, T * BL], F32)

        # ---- attention precompute: pbuf[b,ac,s] = W_pre @ ctx_b + b_pre ----
        for b in range(BL):
            for ac in range(4):
                pp = ps_b.tile([128, S], F32, tag="psb")
                for kc in range(4):
                    nc.tensor.matmul(
                        pp[:],
                        wp[:, (kc * 4 + ac) * 128 : (kc * 4 + ac + 1) * 128],
                        ce[:, (kc * BL + b) * S : (kc * BL + b + 1) * S],
                        start=(kc == 0),
                        stop=(kc == 3),
                    )
                dst = pbuf[:, (b * 4 + ac) * S : (b * 4 + ac + 1) * S]
                if nonzero_bias:
                    nc.scalar.activation(dst, pp[:], AF.Identity, bias=bpre[:, ac : ac + 1])
                elif (b * 4 + ac) % 2 == 0:
                    nc.vector.tensor_copy(dst, pp[:])
                else:
                    nc.scalar.copy(dst, pp[:])

        # ---- helper: readout + copy-gate for step tau (uses h,c AFTER step tau) ----
        def emit_readout(tau, h_t, c_t, after_inst=None):
            psro = ps_r.tile([128, 5, BL], F32)
            for mt in range(4):
                for kc in range(12):
                    if kc < 4:
                        rhs = emb[:, (kc * T + tau) * BL : (kc * T + tau + 1) * BL]
                    elif kc < 8:
                        rhs = h_t[:, (kc - 4) * BL : (kc - 3) * BL]
                    else:
                        rhs = c_t[:, (kc - 8) * BL : (kc - 7) * BL]
                    nc.tensor.matmul(
                        psro[:, mt, :],
                        wr[:, (kc * 4 + mt) * 128 : (kc * 4 + mt + 1) * 128],
                        rhs,
                        start=(kc == 0),
                        stop=(kc == 11),
                    )
            # copy gate: W_copy @ [h; c]
            for kc in range(8):
                rhs = h_t[:, kc * BL : (kc + 1) * BL] if kc < 4 else c_t[:, (kc - 4) * BL : (kc - 3) * BL]
                nc.tensor.matmul(
                    psro[0:1, 4, :],
                    wcp[:, kc : kc + 1],
                    rhs,
                    start=(kc == 0),
                    stop=(kc == 7),
                    skip_group_check=True,
                )
            # maxout: g[jc] = max(ro[jc] (+be), ro[jc+2] (+bo))
            odd = grupool.tile([128, 2, BL], F32)
            ctx_lp = lowprio()
            ctx_lp.__enter__()
            deps = []
            if nonzero_bias:
                for mt in (2, 3):
                    nc.vector.tensor_scalar(
                        out=odd[:, mt - 2, :],
                        in0=psro[:, mt, :],
                        scalar1=brd[:, mt : mt + 1],
                        scalar2=None,
                        op0=ALU.add,
                    )
                ev = grupool.tile([128, 2, BL], F32)
                for mt in (0, 1):
                    nc.vector.tensor_scalar(
                        out=ev[:, mt, :],
                        in0=psro[:, mt, :],
                        scalar1=brd[:, mt : mt + 1],
                        scalar2=None,
                        op0=ALU.add,
                    )
                deps.append(nc.vector.tensor_max(g_buf[:, :, tau, :], ev[:], odd[:]))
            else:
                deps.append(nc.vector.tensor_copy(odd[:], psro[:, 2:4, :]))
                deps.append(nc.vector.tensor_max(g_buf[:, :, tau, :], psro[:, 0:2, :], odd[:]))
            deps.append(nc.vector.tensor_copy(zs_buf[0:1, tau * BL : (tau + 1) * BL], psro[0:1, 4, :]))
            ctx_lp.__exit__(None, None, None)
            if after_inst is not None:
                for dp in deps:
                    bass._add_dep_helper(dp.ins, after_inst.ins, sync=False,
                                         reason="keep off-chain maxout out of the GRU vector window")

        # ---- the scan ----
        # Gate matmuls that do not depend on the attention context (the
        # identity-add of the precomputed embedding part, W_hh@h for r/z and
        # for the n-candidate) are emitted one step early so they execute
        # inside the previous step's tanh window. Only W_ihc@ctx stays on the
        # serial chain.
        def emit_early_gates(psg, h_src, t):
            for mt in range(8):
                dst = psg[:, mt * 8 : mt * 8 + 8]
                nc.tensor.matmul(
                    dst, ident[:], gxe[:, (mt * T + t) * BL : (mt * T + t + 1) * BL],
                    start=True, stop=False, skip_group_check=True,
                )
                for kc in range(4):
                    nc.tensor.matmul(
                        dst, wh[:, (kc * 12 + mt) * 128 : (kc * 12 + mt + 1) * 128],
                        h_src[:, kc * 8 : kc * 8 + 8],
                        start=False, stop=False, skip_group_check=True,
                    )
            for mt in range(8, 12):
                dst = psg[:, 64 + (mt - 8) * 8 : 64 + (mt - 7) * 8]
                nc.tensor.matmul(
                    dst, ident[:], gxe[:, (mt * T + t) * BL : (mt * T + t + 1) * BL],
                    start=True, stop=False, skip_group_check=True,
                )
            for mt in range(8, 12):
                dst = psg[:, 96 + (mt - 8) * 8 : 96 + (mt - 7) * 8]
                for kc in range(4):
                    nc.tensor.matmul(
                        dst, wh[:, (kc * 12 + mt) * 128 : (kc * 12 + mt + 1) * 128],
                        h_src[:, kc * 8 : kc * 8 + 8],
                        start=(kc == 0), stop=(kc == 3), skip_group_check=True,
                    )

        GRPS = [(b,) for b in range(BL)]
        h_t, c_t = h0, c0
        cu_t = c0  # unnormalized == normalized at t=0 (init_att input)
        inv_cur = None
        psg = ps_g.tile([128, 128], F32, tag="psg")
        a_sb_cur = None
        inv_cur = ones_inv
        if early_gates:
            emit_early_gates(psg, h0, 0)
            a_sb_cur = smpool.tile([128, 128], F32, tag="a_sb")
            nc.vector.tensor_copy(a_sb_cur[:], psg[:])
        for t in range(T):
            if not early_gates:
                emit_early_gates(psg, h_t, t)
                a_sb = smpool.tile([128, 128], F32, tag="a_sb")
                nc.vector.tensor_copy(a_sb[:], psg[:])
            else:
                a_sb = a_sb_cur
            # finish gates: the ctx-dependent half (on the serial chain), in its
            # own PSUM bank (a group interleaved with other groups' starts in
            # one bank would be zeroed by the pending-zero region semantics)
            psb = ps_b.tile([128, 96], F32, tag="psb")
            for mt in range(12):
                col = mt * 8
                dst = psb[:, col : col + 8]
                for kc in range(4):
                    nc.tensor.matmul(
                        dst, wx[:, (kc * 12 + mt) * 128 : (kc * 12 + mt + 1) * 128],
                        cu_t[:, kc * 8 : kc * 8 + 8],
                        start=(kc == 0), stop=(kc == 3), skip_group_check=True,
                    )
            # merge A (ident+gh, from the previous tanh window) with B.
            # B used the *unnormalized* attention context, so scale each batch
            # column by the softmax 1/denominator here (scaling commutes
            # through the W_ihc matmul columns).
            sm = grupool.tile([128, 96], F32)
            nc.vector.tensor_mul(
                sm.rearrange("p (mt b) -> p mt b", mt=12)[:],
                psb.rearrange("p (mt b) -> p mt b", mt=12)[:],
                inv_cur.broadcast_to([128, 12, BL]),
            )
            s_rz = grupool.tile([128, 64], F32)
            nc.vector.tensor_add(s_rz[:], a_sb[:, 0:64], sm[:, 0:64])
            s_xn = grupool.tile([128, 32], F32)
            nc.vector.tensor_add(s_xn[:], a_sb[:, 64:96], sm[:, 64:96])

            # GRU elementwise. sigma(x) = .5 + .5 tanh(x/2):
            #   r*hn = .5*(1+t_r)*hn ; z*(h-n) = .5*(1+t_z)*(h-n)
            trz = grupool.tile([128, 64], BF16)
            nc.scalar.activation(trz[:], s_rz[:], AF.Tanh, scale=0.5)
            if nonzero_bias:
                hn = grupool.tile([128, 32], BF16)
                for c in range(4):
                    nc.vector.tensor_scalar(
                        out=hn[:, c * 8 : c * 8 + 8],
                        in0=a_sb[:, 96 + c * 8 : 96 + c * 8 + 8],
                        scalar1=bhhn[:, c : c + 1], scalar2=None, op0=ALU.add,
                    )
                hn_ap = hn[:]
            else:
                hn_ap = a_sb[:, 96:128]
            if stt_gru:
                w2 = grupool.tile([128, 32], BF16)
                nc.vector.scalar_tensor_tensor(
                    out=w2[:], in0=trz[:, 0:32], scalar=1.0, in1=hn_ap,
                    op0=ALU.add, op1=ALU.mult,
                )
                pre_n = grupool.tile([128, 32], BF16)
                nc.vector.scalar_tensor_tensor(
                    out=pre_n[:], in0=w2[:], scalar=0.5, in1=s_xn[:],
                    op0=ALU.mult, op1=ALU.add,
                )
                n_sb = grupool.tile([128, 32], BF16)
                nc.scalar.activation(n_sb[:], pre_n[:], AF.Tanh)
                dd = grupool.tile([128, 32], BF16)
                nc.vector.tensor_sub(dd[:], h_t[:], n_sb[:])
                v2 = grupool.tile([128, 32], BF16)
                nc.vector.scalar_tensor_tensor(
                    out=v2[:], in0=trz[:, 32:64], scalar=1.0, in1=dd[:],
                    op0=ALU.add, op1=ALU.mult,
                )
                h_new = hpool.tile([128, 32], BF16)
                h_new_inst = nc.vector.scalar_tensor_tensor(
                    out=h_new[:], in0=v2[:], scalar=0.5, in1=n_sb[:],
                    op0=ALU.mult, op1=ALU.add,
                )
            else:
                hn = grupool.tile([128, 32], BF16)
                nc.vector.tensor_copy(hn[:], a_sb[:, 96:128])
                u = grupool.tile([128, 32], BF16)
                nc.vector.tensor_mul(u[:], trz[:, 0:32], hn[:])
                w2 = grupool.tile([128, 32], BF16)
                nc.vector.tensor_add(w2[:], u[:], hn[:])
                pre_n = grupool.tile([128, 32], BF16)
                nc.vector.scalar_tensor_tensor(
                    out=pre_n[:], in0=w2[:], scalar=0.5, in1=s_xn[:],
                    op0=ALU.mult, op1=ALU.add,
                )
                n_sb = grupool.tile([128, 32], BF16)
                nc.scalar.activation(n_sb[:], pre_n[:], AF.Tanh)
                dd = grupool.tile([128, 32], BF16)
                nc.vector.tensor_sub(dd[:], h_t[:], n_sb[:])
                v1t = grupool.tile([128, 32], BF16)
                nc.vector.tensor_mul(v1t[:], trz[:, 32:64], dd[:])
                v2 = grupool.tile([128, 32], BF16)
                nc.vector.tensor_add(v2[:], v1t[:], dd[:])
                h_new = hpool.tile([128, 32], BF16)
                nc.vector.scalar_tensor_tensor(
                    out=h_new[:], in0=v2[:], scalar=0.5, in1=n_sb[:],
                    op0=ALU.mult, op1=ALU.add,
                )

            if debug_t0 and t == 0:
                dbgp = smpool.tile([128, 128], F32, tag="dbgp")
                nc.vector.tensor_copy(dbgp[:], psg[:])
                nc.sync.dma_start(out=d_dbg_psg, in_=dbgp[:])
                dbgh = smpool.tile([128, 32], F32, tag="dbgh")
                nc.vector.tensor_copy(dbgh[:], h_new[:])
                nc.sync.dma_start(out=d_dbg_h, in_=dbgh[:])

            # target = W_q @ h_new
            ptgt = ps_t.tile([128, 32], F32)
            for ac in range(4):
                for kc in range(4):
                    nc.tensor.matmul(
                        ptgt[:, ac * 8 : ac * 8 + 8],
                        wq[:, (kc * 4 + ac) * 128 : (kc * 4 + ac + 1) * 128],
                        h_new[:, kc * 8 : kc * 8 + 8],
                        start=(kc == 0), stop=(kc == 3), skip_group_check=True,
                    )

            # next step's context-independent gate matmuls fill the tanh window
            if t + 1 < T:
                psg_next = ps_g.tile([128, 128], F32, tag="psg")
                if early_gates:
                    emit_early_gates(psg_next, h_new, t + 1)
            if t == T - 1:
                hidf = smpool.tile([128, 32], F32)
                nc.vector.tensor_copy(hidf[:], h_new[:])
                nc.sync.dma_start(out=d_hid, in_=hidf[:])

            if psum_tgt:
                tgt_src = ptgt
            else:
                tgt_src = smpool.tile([128, 32], F32, tag="tgtsb")
                nc.vector.tensor_copy(tgt_src[:], ptgt[:])

            # previous step's readout also runs in this step's tanh window
            if t > 0:
                emit_readout(t - 1, h_t, c_t, after_inst=h_new_inst)

            # attention
            pse = ps_e.tile([128, 2, BL], F32)
            exps = smpool.tile([128, 2, BL], BF16)
            psc = ps_c.tile([128, 4, BL], F32)

            def emit_exp_pair(b1, pre_ctx=None):
                if nonzero_mask:
                    me = smpool.tile([128, 2, 2], F32)
                    nc.vector.tensor_mul(
                        me[:], pse[:, :, b1 - 1 : b1 + 1],
                        ma.rearrange("p (sc b) -> p sc b", sc=2)[:, :, b1 - 1 : b1 + 1],
                    )
                    nc.vector.tensor_add(
                        me[:], me[:],
                        mc.rearrange("p (sc b) -> p sc b", sc=2)[:, :, b1 - 1 : b1 + 1],
                    )
                    nc.scalar.activation(exps[:, :, b1 - 1 : b1 + 1], me[:], AF.Exp)
                else:
                    nc.scalar.activation(
                        exps[:, :, b1 - 1 : b1 + 1], pse[:, :, b1 - 1 : b1 + 1], AF.Exp
                    )
                if pre_ctx is not None:
                    pre_ctx()
                for bb in (b1 - 1, b1):
                    for ec in range(4):
                        for sc in range(2):
                            nc.tensor.matmul(
                                psc[:, ec, bb : bb + 1],
                                ct[:, ((bb * 2 + sc) * 4 + ec) * 128 : ((bb * 2 + sc) * 4 + ec + 1) * 128],
                                exps[:, sc, bb : bb + 1],
                                start=(sc == 0), stop=(sc == 1), skip_group_check=True,
                            )

            for grp in GRPS:
                glen = len(grp)
                tmpg = tmppool.tile([128, glen * 4 * S], BF16, tag="tmpg")
                for j, b in enumerate(grp):
                    for ac in range(4):
                        last_add_inst = nc.vector.tensor_scalar(
                            out=tmpg[:, (j * 4 + ac) * S : (j * 4 + ac + 1) * S],
                            in0=pbuf[:, (b * 4 + ac) * S : (b * 4 + ac + 1) * S],
                            scalar1=tgt_src[:, ac * 8 + b : ac * 8 + b + 1],
                            scalar2=None, op0=ALU.add,
                        )
                thg = thpool.tile([128, glen * 4 * S], BF16, tag="thg")
                if grp[0] == 0:
                    nc.scalar.activation(thg[:, 0 : 2 * S], tmpg[:, 0 : 2 * S], AF.Tanh)
                    nc.scalar.activation(thg[:, 2 * S :], tmpg[:, 2 * S :], AF.Tanh)
                else:
                    nc.scalar.activation(thg[:], tmpg[:], AF.Tanh)
                if debug_t0 and t == 0 and grp[0] == 0:
                    dbgt = smpool.tile([128, 32], F32, tag="dbgt")
                    nc.vector.tensor_copy(dbgt[:], ptgt[:])
                    nc.sync.dma_start(out=d_dbg_tgt, in_=dbgt[:])
                    dbgth = smpool.tile([128, 4 * S], F32, tag="dbgth")
                    nc.vector.tensor_copy(dbgth[:], thg[:, 0 : 4 * S])
                    nc.sync.dma_start(out=d_dbg_th, in_=dbgth[:])
                for j, b in enumerate(grp):
                    for sc in range(2):
                        for ac in range(4):
                            nc.tensor.matmul(
                                pse[:, sc, b : b + 1],
                                thg[:, (j * 4 + ac) * S + sc * 128 : (j * 4 + ac) * S + sc * 128 + 128],
                                va[:, ac : ac + 1],
                                start=(ac == 0), stop=(ac == 3), skip_group_check=True,
                            )
                    if b % 2 == 1:
                        if b < 7:
                            emit_exp_pair(b)

            # last pair: denominator + reciprocal emitted right after the exp,
            # overlapping the final newctx matmuls
            psm = ps_m.tile([128, 16], F32)
            inv = smpool.tile([1, 8], BF16)

            def den_emit():
                nc.tensor.matmul(psm[0:1, 0:8], onesc[:], exps[:, 0, :], start=True, stop=False, skip_group_check=True)
                nc.tensor.matmul(psm[0:1, 0:8], onesc[:], exps[:, 1, :], start=False, stop=True, skip_group_check=True)
                with nc.allow_low_precision(reason="softmax 1/denom in bf16 is enough"):
                    nc.vector.reciprocal(inv[:], psm[0:1, 0:8])

            emit_exp_pair(7, pre_ctx=den_emit)
            nc.tensor.matmul(psm[:, 8:16], onesr[:], inv[:], start=True, stop=True, skip_group_check=True)
            invbc = smpool.tile([128, 1, BL], F32)
            nc.vector.tensor_copy(invbc[:, 0, :], psm[:, 8:16])

            # unnormalized context for the next gx (normalization folded into
            # the gate merge); available right after the newctx matmuls
            cu_new = cxpool.tile([128, 32], BF16, tag="cu")
            nc.vector.tensor_copy(cu_new[:], psc.rearrange("p ec b -> p (ec b)")[:])
            # normalized context for readout/copy-gate (off the critical chain)
            c_new = cxpool.tile([128, 32], BF16)
            c3 = c_new.rearrange("p (ec b) -> p ec b", ec=4)
            with lowprio():
                if bcast:
                    cn_inst = nc.vector.tensor_mul(c3[:], psc[:], invbc.broadcast_to([128, 4, BL]))
                else:
                    for ec in range(4):
                        cn_inst = nc.vector.tensor_mul(c3[:, ec, :], psc[:, ec, :], invbc[:, 0, :])
            with lowprio():
                for sc in range(2):
                    nc.vector.tensor_mul(co_buf[:, sc, t, :], exps[:, sc, :], invbc[:, 0, :])

            if t == T - 1:
                cctxf = smpool.tile([128, 32], F32)
                cc3 = cctxf.rearrange("p (ec b) -> p ec b", ec=4)
                if bcast:
                    nc.vector.tensor_mul(cc3[:], psc[:], invbc.broadcast_to([128, 4, BL]))
                else:
                    for ec in range(4):
                        nc.vector.tensor_mul(cc3[:, ec, :], psc[:, ec, :], invbc[:, 0, :])
                nc.sync.dma_start(out=d_cctx, in_=cctxf[:])

            h_t, c_t = h_new, c_new
            cu_t, inv_cur = cu_new, invbc
            if t + 1 < T:
                psg = psg_next
                if early_gates:
                    # this PSUM->SBUF copy waits on the PE early-gate matmuls;
                    # force it after the attention adds so it cannot stall them
                    a_sb_next = smpool.tile([128, 128], F32, tag="a_sb")
                    with lowprio():
                        cp = nc.vector.tensor_copy(a_sb_next[:], psg_next[:])
                    bass._add_dep_helper(cp.ins, last_add_inst.ins, sync=False,
                                         reason="a_sb copy after attention adds")
                    a_sb_cur = a_sb_next

        # final readout (tau = T-1) and copy-gate activation
        emit_readout(T - 1, h_t, c_t)
        zst = cpool.tile([1, T * BL], F32)
        nc.scalar.activation(zst[:], zs_buf[:], AF.Tanh, scale=0.5, bias=0.5 * b_copy_val)
        zso = cpool.tile([1, T * BL], F32)
        nc.vector.tensor_scalar(
            out=zso[:], in0=zst[:], scalar1=0.5, scalar2=0.5, op0=ALU.mult, op1=ALU.add
        )
        nc.sync.dma_start(out=d_zs, in_=zso[:])
        nc.sync.dma_start(out=d_g, in_=g_buf.rearrange("p a t b -> p (a t b)"))
        nc.sync.dma_start(out=d_co, in_=co_buf.rearrange("p a t b -> p (a t b)"))

    nc.compile()
    return nc


# --------------------------------------------------------------------------
# Host-side packing
# --------------------------------------------------------------------------
def _bf(x):
    return np.ascontiguousarray(x.astype(ml_dtypes.bfloat16))


def _f32(x):
    return np.ascontiguousarray(x.astype(np.float32))


def _prep_inputs(input_ids, hidden, context, src_pad_mask, init_att, emb_table,
                 W_ih, W_hh, b_ih, b_hh, W_pre, b_pre, W_q, v_att,
                 W_copy, b_copy, W_read, b_read):
    nonzero_mask = bool(np.any(src_pad_mask))
    nonzero_bias = bool(
        np.any(b_pre) or np.any(b_hh[1024:]) or np.any(b_read)
    )

    emb = emb_table[input_ids]  # [T, B, WV] gather on host
    bias_fold = b_ih.astype(np.float64).copy()
    bias_fold[:1024] += b_hh[:1024]
    gxe_full = emb.astype(np.float32) @ W_ih[:, :WV].T + bias_fold.astype(np.float32)

    perm = np.concatenate([np.arange(0, DEC, 2), np.arange(1, DEC, 2)])
    wrp = W_read[perm]
    brd_p = b_read[perm]

    def pack_w(wT_src, nm):  # [nm*128, nk*128] -> [128,(kc,mt,m)]
        nk = wT_src.shape[1] // 128
        return _bf(
            wT_src.reshape(nm, 128, nk, 128).transpose(3, 2, 0, 1).reshape(128, -1)
        )

    shared = {
        "wh": pack_w(W_hh, 12),
        "wx": pack_w(W_ih[:, WV:], 12),
        "wq": pack_w(W_q, 4),
        "wr": pack_w(wrp, 4),
        "wp": pack_w(W_pre, 4),
        "wcp": _bf(W_copy.reshape(8, 128).T),
        "va": _bf(v_att.reshape(4, 128).T),
        "ident": _bf(np.eye(128, dtype=np.float32)),
        "onesc": _bf(np.ones((128, 1), np.float32)),
        "onesr": _bf(np.ones((1, 128), np.float32)),
    }
    if nonzero_bias:
        shared["bpre"] = _f32(b_pre.reshape(4, 128).T)
        shared["bhhn"] = _f32(b_hh[1024:].reshape(4, 128).T)
        shared["brd"] = _f32(brd_p.reshape(4, 128).T)

    in_maps = []
    for core in range(NCORES):
        bg = core * BL
        m = dict(shared)
        m["gxe"] = _bf(
            gxe_full[:, bg : bg + BL].reshape(T, BL, 12, 128).transpose(3, 2, 0, 1).reshape(128, -1)
        )
        m["emb"] = _bf(
            emb[:, bg : bg + BL].reshape(T, BL, 4, 128).transpose(3, 2, 0, 1).reshape(128, -1)
        )
        m["ce"] = _bf(
            context[:, bg : bg + BL].reshape(S, BL, 4, 128).transpose(3, 2, 1, 0).reshape(128, -1)
        )
        m["ct"] = _bf(
            context.reshape(2, 128, B, 4, 128)[:, :, bg : bg + BL]
            .transpose(1, 2, 0, 3, 4)
            .reshape(128, -1)
        )
        m["h0"] = _bf(hidden[0, bg : bg + BL].reshape(BL, 4, 128).transpose(2, 1, 0).reshape(128, -1))
        m["c0"] = _bf(init_att[bg : bg + BL].reshape(BL, 4, 128).transpose(2, 1, 0).reshape(128, -1))
        if nonzero_mask:
            msk = src_pad_mask[bg : bg + BL].astype(np.float32)  # [BL, S]
            ma = (1.0 - msk).T.reshape(2, 128, BL).transpose(1, 0, 2).reshape(128, -1)
            mcv = (-1e6 * msk).T.reshape(2, 128, BL).transpose(1, 0, 2).reshape(128, -1)
            m["ma"] = _f32(ma)
            m["mc"] = _f32(mcv)
        in_maps.append(m)
    return in_maps, nonzero_mask, nonzero_bias, float(b_copy[0])


def _unpack(results):
    g_parts, c_parts, z_parts, h_parts, cc_parts = [], [], [], [], []
    for r in results:
        g_parts.append(
            r["g"].reshape(128, 2, T, BL).transpose(2, 3, 1, 0).reshape(T, BL, 256)
        )
        c_parts.append(
            r["co"].astype(np.float32).reshape(128, 2, T, BL).transpose(2, 3, 1, 0).reshape(T, BL, S)
        )
        z_parts.append(r["zs"].reshape(T, BL, 1))
        h_parts.append(r["hid"].reshape(128, 4, BL).transpose(2, 1, 0).reshape(BL, DEC))
        cc_parts.append(r["cctx"].reshape(128, 4, BL).transpose(2, 1, 0).reshape(BL, ENC))
    g_out = np.concatenate(g_parts, axis=1).astype(np.float32)
    c_out = np.concatenate(c_parts, axis=1).astype(np.float32)
    copy_out = np.concatenate(z_parts, axis=1).astype(np.float32)
    hid = np.concatenate(h_parts, axis=0)[None].astype(np.float32)
    cur_ctx = np.concatenate(cc_parts, axis=0).astype(np.float32)
    return g_out, c_out, copy_out, hid, c_out[-1], cur_ctx


def kernel(**inputs):
    inputs = {k: np.asarray(v) for k, v in inputs.items()}
    in_maps, nzm, nzb, b_copy_val = _prep_inputs(**inputs)
    key = (nzm, nzb, b_copy_val)
    if key not in _CACHE:
        _CACHE[key] = _build(nzm, nzb, b_copy_val)
    nc = _CACHE[key]
    res = run_bass_kernel_spmd(nc, in_maps, list(range(NCORES)))
    return _unpack(res.results)


# revision 17
# speedup vs baseline: 1.1229x; 1.1229x over previous
"""Trainium2 Bass kernel for an input-feed GRU decoder with concat attention,
copy gate, and maxout readout.

Sharding: data-parallel over batch B=64 across 8 NeuronCores (8 batch rows per
core, params replicated). The T=32 recurrence is sequential; each core runs the
full scan for its batch slice with zero cross-core communication.

Layouts are feature-major: [feature-chunk-on-partitions, batch-on-free].
All matmuls run in bf16 (fp32 PSUM accumulate). Sigmoid is computed as
0.5 + 0.5*tanh(x/2) so the whole kernel uses one activation table set
(exp_and_others: tanh + exp + copy/identity).
"""

import sys

sys.path.insert(0, "/opt/trn_rl_repo")

from contextlib import ExitStack

import ml_dtypes
import numpy as np

import concourse.bass as bass
import concourse.tile as tile
from concourse import bacc, mybir
from concourse.bass_utils import run_bass_kernel_spmd

BF16 = mybir.dt.bfloat16
F32 = mybir.dt.float32
AF = mybir.ActivationFunctionType
ALU = mybir.AluOpType

# Problem shapes (hardcoded per contest rules).
T, B, S = 32, 64, 256
DEC = ENC = ATT = WV = 512
NCORES = 8
BL = B // NCORES  # 8 local batch
NMT = 12  # gate row tiles (3*DEC/128)
NKC = 4  # 512/128

_CACHE = {}


# --------------------------------------------------------------------------
# Device program
# --------------------------------------------------------------------------
def _build(nonzero_mask: bool, nonzero_bias: bool, b_copy_val: float,
           early_gates=True, psum_tgt=False, stt_gru=True, bcast=True, debug_t0=False):
    nc = bacc.Bacc("TRN2", target_bir_lowering=False, debug=False, num_devices=NCORES)

    def din(name, shape, dt=BF16):
        return nc.dram_tensor(name, shape, dt, kind="ExternalInput").ap()

    def dout(name, shape, dt=F32):
        return nc.dram_tensor(name, shape, dt, kind="ExternalOutput").ap()

    d_wh = din("wh", [128, 4 * 12 * 128])
    d_wx = din("wx", [128, 4 * 12 * 128])
    d_wq = din("wq", [128, 4 * 4 * 128])
    d_wr = din("wr", [128, 12 * 4 * 128])
    d_wp = din("wp", [128, 4 * 4 * 128])
    d_wcp = din("wcp", [128, 8])
    d_va = din("va", [128, 4])
    d_gxe = din("gxe", [128, 12 * T * BL])
    d_emb = din("emb", [128, 4 * T * BL])
    d_ce = din("ce", [128, 4 * BL * S])
    d_ct = din("ct", [128, BL * 2 * 4 * 128])
    d_h0 = din("h0", [128, 32])
    d_c0 = din("c0", [128, 32])
    d_ident = din("ident", [128, 128])
    d_onesc = din("onesc", [128, 1])
    d_onesr = din("onesr", [1, 128])
    if nonzero_mask:
        d_ma = din("ma", [128, 2 * BL], F32)
        d_mc = din("mc", [128, 2 * BL], F32)
    if nonzero_bias:
        d_bpre = din("bpre", [128, 4], F32)
        d_bhhn = din("bhhn", [128, 4], F32)
        d_brd = din("brd", [128, 4], F32)

    if debug_t0:
        d_dbg_psg = dout("dbg_psg", [128, 128])
        d_dbg_h = dout("dbg_h", [128, 32])
        d_dbg_tgt = dout("dbg_tgt", [128, 32])
        d_dbg_th = dout("dbg_th", [128, 4 * S])
    d_g = dout("g", [128, 2 * T * BL])
    d_co = dout("co", [128, 2 * T * BL], BF16)
    d_zs = dout("zs", [1, T * BL])
    d_hid = dout("hid", [128, 32])
    d_cctx = dout("cctx", [128, 32])

    with tile.TileContext(nc) as tc, ExitStack() as ctx:
        from contextlib import contextmanager

        @contextmanager
        def lowprio(bump=1_000_000):
            tc.cur_priority += bump
            try:
                yield
            finally:
                tc.cur_priority -= bump

        cpool = ctx.enter_context(tc.tile_pool(name="consts", bufs=1))
        hpool = ctx.enter_context(tc.tile_pool(name="hstate", bufs=2))
        cxpool = ctx.enter_context(tc.tile_pool(name="cstate", bufs=2))
        tmppool = ctx.enter_context(tc.tile_pool(name="tmp", bufs=3))
        thpool = ctx.enter_context(tc.tile_pool(name="th", bufs=3))
        grupool = ctx.enter_context(tc.tile_pool(name="gru", bufs=2))
        smpool = ctx.enter_context(tc.tile_pool(name="smx", bufs=2))

        ps_g = ctx.enter_context(tc.tile_pool(name="psg", bufs=2, space="PSUM"))
        ps_t = ctx.enter_context(tc.tile_pool(name="pst", bufs=1, space="PSUM"))
        ps_e = ctx.enter_context(tc.tile_pool(name="pse", bufs=1, space="PSUM"))
        ps_c = ctx.enter_context(tc.tile_pool(name="psc", bufs=1, space="PSUM"))
        ps_r = ctx.enter_context(tc.tile_pool(name="psr", bufs=1, space="PSUM"))
        ps_m = ctx.enter_context(tc.tile_pool(name="psm", bufs=1, space="PSUM"))
        ps_b = ctx.enter_context(tc.tile_pool(name="psb", bufs=1, space="PSUM"))

        # ---- constant loads ----
        def cload(dram, shape, dt=BF16):
            t_ = cpool.tile(shape, dt, tag=dram.tensor.name)
            nc.sync.dma_start(out=t_, in_=dram)
            return t_

        h0 = cload(d_h0, [128, 32])
        c0 = cload(d_c0, [128, 32])
        gxe = cload(d_gxe, [128, 12 * T * BL])
        wh = cload(d_wh, [128, 4 * 12 * 128])
        wx = cload(d_wx, [128, 4 * 12 * 128])
        ident = cload(d_ident, [128, 128])
        wp = cload(d_wp, [128, 4 * 4 * 128])
        ce = cload(d_ce, [128, 4 * BL * S])
        wq = cload(d_wq, [128, 4 * 4 * 128])
        va = cload(d_va, [128, 4])
        ct = cload(d_ct, [128, BL * 2 * 4 * 128])
        wr = cload(d_wr, [128, 12 * 4 * 128])
        wcp = cload(d_wcp, [128, 8])
        emb = cload(d_emb, [128, 4 * T * BL])
        onesc = cload(d_onesc, [128, 1])
        onesr = cload(d_onesr, [1, 128])
        ma = cload(d_ma, [128, 2 * BL], F32) if nonzero_mask else None
        mc = cload(d_mc, [128, 2 * BL], F32) if nonzero_mask else None
        bpre = cload(d_bpre, [128, 4], F32) if nonzero_bias else None
        bhhn = cload(d_bhhn, [128, 4], F32) if nonzero_bias else None
        brd = cload(d_brd, [128, 4], F32) if nonzero_bias else None

        ones_inv = cpool.tile([128, 1, BL], F32)
        nc.vector.memset(ones_inv[:], 1.0)
        pbuf = cpool.tile([128, BL * 4 * S], BF16)  # tanh arg precompute
        g_buf = cpool.tile([128, 2, T, BL], F32)
        co_buf = cpool.tile([128, 2, T, BL], BF16)
        zs_buf = cpool.tile([1, T * BL], F32)

        # ---- attention precompute: pbuf[b,ac,s] = W_pre @ ctx_b + b_pre ----
        for b in range(BL):
            for ac in range(4):
                pp = ps_b.tile([128, S], F32, tag="psb")
                for kc in range(4):
                    nc.tensor.matmul(
                        pp[:],
                        wp[:, (kc * 4 + ac) * 128 : (kc * 4 + ac + 1) * 128],
                        ce[:, (kc * BL + b) * S : (kc * BL + b + 1) * S],
                        start=(kc == 0),
                        stop=(kc == 3),
                    )
                dst = pbuf[:, (b * 4 + ac) * S : (b * 4 + ac + 1) * S]
                if nonzero_bias:
                    nc.scalar.activation(dst, pp[:], AF.Identity, bias=bpre[:, ac : ac + 1])
                elif (b * 4 + ac) % 2 == 0:
                    nc.vector.tensor_copy(dst, pp[:])
                else:
                    nc.scalar.copy(dst, pp[:])

        # ---- helper: readout + copy-gate for step tau (uses h,c AFTER step tau) ----
        def emit_readout(tau, h_t, c_t, after_inst=None):
            psro = ps_r.tile([128, 5, BL], F32)
            for mt in range(4):
                for kc in range(12):
                    if kc < 4:
                        rhs = emb[:, (kc * T + tau) * BL : (kc * T + tau + 1) * BL]
                    elif kc < 8:
                        rhs = h_t[:, (kc - 4) * BL : (kc - 3) * BL]
                    else:
                        rhs = c_t[:, (kc - 8) * BL : (kc - 7) * BL]
                    nc.tensor.matmul(
                        psro[:, mt, :],
                        wr[:, (kc * 4 + mt) * 128 : (kc * 4 + mt + 1) * 128],
                        rhs,
                        start=(kc == 0),
                        stop=(kc == 11),
                    )
            # copy gate: W_copy @ [h; c]
            for kc in range(8):
                rhs = h_t[:, kc * BL : (kc + 1) * BL] if kc < 4 else c_t[:, (kc - 4) * BL : (kc - 3) * BL]
                nc.tensor.matmul(
                    psro[0:1, 4, :],
                    wcp[:, kc : kc + 1],
                    rhs,
                    start=(kc == 0),
                    stop=(kc == 7),
                    skip_group_check=True,
                )
            # maxout: g[jc] = max(ro[jc] (+be), ro[jc+2] (+bo))
            odd = grupool.tile([128, 2, BL], F32)
            ctx_lp = lowprio()
            ctx_lp.__enter__()
            deps = []
            if nonzero_bias:
                for mt in (2, 3):
                    nc.vector.tensor_scalar(
                        out=odd[:, mt - 2, :],
                        in0=psro[:, mt, :],
                        scalar1=brd[:, mt : mt + 1],
                        scalar2=None,
                        op0=ALU.add,
                    )
                ev = grupool.tile([128, 2, BL], F32)
                for mt in (0, 1):
                    nc.vector.tensor_scalar(
                        out=ev[:, mt, :],
                        in0=psro[:, mt, :],
                        scalar1=brd[:, mt : mt + 1],
                        scalar2=None,
                        op0=ALU.add,
                    )
                deps.append(nc.vector.tensor_max(g_buf[:, :, tau, :], ev[:], odd[:]))
            else:
                deps.append(nc.vector.tensor_copy(odd[:], psro[:, 2:4, :]))
                deps.append(nc.vector.tensor_max(g_buf[:, :, tau, :], psro[:, 0:2, :], odd[:]))
            deps.append(nc.vector.tensor_copy(zs_buf[0:1, tau * BL : (tau + 1) * BL], psro[0:1, 4, :]))
            ctx_lp.__exit__(None, None, None)
            if after_inst is not None:
                for dp in deps:
                    bass._add_dep_helper(dp.ins, after_inst.ins, sync=False,
                                         reason="keep off-chain maxout out of the GRU vector window")

        # ---- the scan ----
        # Gate matmuls that do not depend on the attention context (the
        # identity-add of the precomputed embedding part, W_hh@h for r/z and
        # for the n-candidate) are emitted one step early so they execute
        # inside the previous step's tanh window. Only W_ihc@ctx stays on the
        # serial chain.
        def emit_early_gates(psg, h_src, t):
            for mt in range(8):
                dst = psg[:, mt * 8 : mt * 8 + 8]
                nc.tensor.matmul(
                    dst, ident[:], gxe[:, (mt * T + t) * BL : (mt * T + t + 1) * BL],
                    start=True, stop=False, skip_group_check=True,
                )
                for kc in range(4):
                    nc.tensor.matmul(
                        dst, wh[:, (kc * 12 + mt) * 128 : (kc * 12 + mt + 1) * 128],
                        h_src[:, kc * 8 : kc * 8 + 8],
                        start=False, stop=False, skip_group_check=True,
                    )
            for mt in range(8, 12):
                dst = psg[:, 64 + (mt - 8) * 8 : 64 + (mt - 7) * 8]
                nc.tensor.matmul(
                    dst, ident[:], gxe[:, (mt * T + t) * BL : (mt * T + t + 1) * BL],
                    start=True, stop=False, skip_group_check=True,
                )
            for mt in range(8, 12):
                dst = psg[:, 96 + (mt - 8) * 8 : 96 + (mt - 7) * 8]
                for kc in range(4):
                    nc.tensor.matmul(
                        dst, wh[:, (kc * 12 + mt) * 128 : (kc * 12 + mt + 1) * 128],
                        h_src[:, kc * 8 : kc * 8 + 8],
                        start=(kc == 0), stop=(kc == 3), skip_group_check=True,
                    )

        GRPS = [(b,) for b in range(BL)]
        h_t, c_t = h0, c0
        cu_t = c0  # unnormalized == normalized at t=0 (init_att input)
        inv_cur = None
        psg = ps_g.tile([128, 128], F32, tag="psg")
        a_sb_cur = None
        inv_cur = ones_inv
        if early_gates:
            emit_early_gates(psg, h0, 0)
            a_sb_cur = smpool.tile([128, 128], F32, tag="a_sb")
            nc.vector.tensor_copy(a_sb_cur[:], psg[:])
        for t in range(T):
            if not early_gates:
                emit_early_gates(psg, h_t, t)
                a_sb = smpool.tile([128, 128], F32, tag="a_sb")
                nc.vector.tensor_copy(a_sb[:], psg[:])
            else:
                a_sb = a_sb_cur
            # finish gates: the ctx-dependent half (on the serial chain), in its
            # own PSUM bank (a group interleaved with other groups' starts in
            # one bank would be zeroed by the pending-zero region semantics)
            psb = ps_b.tile([128, 96], F32, tag="psb")
            for mt in range(12):
                col = mt * 8
                dst = psb[:, col : col + 8]
                for kc in range(4):
                    nc.tensor.matmul(
                        dst, wx[:, (kc * 12 + mt) * 128 : (kc * 12 + mt + 1) * 128],
                        cu_t[:, kc * 8 : kc * 8 + 8],
                        start=(kc == 0), stop=(kc == 3), skip_group_check=True,
                    )
            # merge A (ident+gh, from the previous tanh window) with B.
            # B used the *unnormalized* attention context, so scale each batch
            # column by the softmax 1/denominator here (scaling commutes
            # through the W_ihc matmul columns).
            sm = grupool.tile([128, 96], F32)
            nc.vector.tensor_mul(
                sm.rearrange("p (mt b) -> p mt b", mt=12)[:],
                psb.rearrange("p (mt b) -> p mt b", mt=12)[:],
                inv_cur.broadcast_to([128, 12, BL]),
            )
            s_rz = grupool.tile([128, 64], F32)
            nc.vector.tensor_add(s_rz[:], a_sb[:, 0:64], sm[:, 0:64])
            s_xn = grupool.tile([128, 32], F32)
            nc.vector.tensor_add(s_xn[:], a_sb[:, 64:96], sm[:, 64:96])

            # GRU elementwise. sigma(x) = .5 + .5 tanh(x/2):
            #   r*hn = .5*(1+t_r)*hn ; z*(h-n) = .5*(1+t_z)*(h-n)
            trz = grupool.tile([128, 64], BF16)
            nc.scalar.activation(trz[:], s_rz[:], AF.Tanh, scale=0.5)
            if nonzero_bias:
                hn = grupool.tile([128, 32], BF16)
                for c in range(4):
                    nc.vector.tensor_scalar(
                        out=hn[:, c * 8 : c * 8 + 8],
                        in0=a_sb[:, 96 + c * 8 : 96 + c * 8 + 8],
                        scalar1=bhhn[:, c : c + 1], scalar2=None, op0=ALU.add,
                    )
                hn_ap = hn[:]
            else:
                hn_ap = a_sb[:, 96:128]
            if stt_gru:
                w2 = grupool.tile([128, 32], BF16)
                nc.vector.scalar_tensor_tensor(
                    out=w2[:], in0=trz[:, 0:32], scalar=1.0, in1=hn_ap,
                    op0=ALU.add, op1=ALU.mult,
                )
                pre_n = grupool.tile([128, 32], BF16)
                nc.vector.scalar_tensor_tensor(
                    out=pre_n[:], in0=w2[:], scalar=0.5, in1=s_xn[:],
                    op0=ALU.mult, op1=ALU.add,
                )
                n_sb = grupool.tile([128, 32], BF16)
                nc.scalar.activation(n_sb[:], pre_n[:], AF.Tanh)
                dd = grupool.tile([128, 32], BF16)
                nc.vector.tensor_sub(dd[:], h_t[:], n_sb[:])
                v2 = grupool.tile([128, 32], BF16)
                nc.vector.scalar_tensor_tensor(
                    out=v2[:], in0=trz[:, 32:64], scalar=1.0, in1=dd[:],
                    op0=ALU.add, op1=ALU.mult,
                )
                h_new = hpool.tile([128, 32], BF16)
                h_new_inst = nc.vector.scalar_tensor_tensor(
                    out=h_new[:], in0=v2[:], scalar=0.5, in1=n_sb[:],
                    op0=ALU.mult, op1=ALU.add,
                )
            else:
                hn = grupool.tile([128, 32], BF16)
                nc.vector.tensor_copy(hn[:], a_sb[:, 96:128])
                u = grupool.tile([128, 32], BF16)
                nc.vector.tensor_mul(u[:], trz[:, 0:32], hn[:])
                w2 = grupool.tile([128, 32], BF16)
                nc.vector.tensor_add(w2[:], u[:], hn[:])
                pre_n = grupool.tile([128, 32], BF16)
                nc.vector.scalar_tensor_tensor(
                    out=pre_n[:], in0=w2[:], scalar=0.5, in1=s_xn[:],
                    op0=ALU.mult, op1=ALU.add,
                )
                n_sb = grupool.tile([128, 32], BF16)
                nc.scalar.activation(n_sb[:], pre_n[:], AF.Tanh)
                dd = grupool.tile([128, 32], BF16)
                nc.vector.tensor_sub(dd[:], h_t[:], n_sb[:])
                v1t = grupool.tile([128, 32], BF16)
                nc.vector.tensor_mul(v1t[:], trz[:, 32:64], dd[:])
                v2 = grupool.tile([128, 32], BF16)
                nc.vector.tensor_add(v2[:], v1t[:], dd[:])
                h_new = hpool.tile([128, 32], BF16)
                nc.vector.scalar_tensor_tensor(
                    out=h_new[:], in0=v2[:], scalar=0.5, in1=n_sb[:],
                    op0=ALU.mult, op1=ALU.add,
                )

            if debug_t0 and t == 0:
                dbgp = smpool.tile([128, 128], F32, tag="dbgp")
                nc.vector.tensor_copy(dbgp[:], psg[:])
                nc.sync.dma_start(out=d_dbg_psg, in_=dbgp[:])
                dbgh = smpool.tile([128, 32], F32, tag="dbgh")
                nc.vector.tensor_copy(dbgh[:], h_new[:])
                nc.sync.dma_start(out=d_dbg_h, in_=dbgh[:])

            # target = W_q @ h_new
            ptgt = ps_t.tile([128, 32], F32)
            for ac in range(4):
                for kc in range(4):
                    nc.tensor.matmul(
                        ptgt[:, ac * 8 : ac * 8 + 8],
                        wq[:, (kc * 4 + ac) * 128 : (kc * 4 + ac + 1) * 128],
                        h_new[:, kc * 8 : kc * 8 + 8],
                        start=(kc == 0), stop=(kc == 3), skip_group_check=True,
                    )

            # next step's context-independent gate matmuls fill the tanh window
            if t + 1 < T:
                psg_next = ps_g.tile([128, 128], F32, tag="psg")
                if early_gates:
                    emit_early_gates(psg_next, h_new, t + 1)
            if t == T - 1:
                hidf = smpool.tile([128, 32], F32)
                nc.vector.tensor_copy(hidf[:], h_new[:])
                nc.sync.dma_start(out=d_hid, in_=hidf[:])

            if psum_tgt:
                tgt_src = ptgt
            else:
                tgt_src = smpool.tile([128, 32], F32, tag="tgtsb")
                nc.vector.tensor_copy(tgt_src[:], ptgt[:])

            # previous step's readout also runs in this step's tanh window
            if t > 0:
                emit_readout(t - 1, h_t, c_t, after_inst=h_new_inst)

            # attention
            pse = ps_e.tile([128, 2, BL], F32)
            exps = smpool.tile([128, 2, BL], BF16)
            psc = ps_c.tile([128, 4, BL], F32)

            def emit_exp_pair(b1, pre_ctx=None):
                if nonzero_mask:
                    me = smpool.tile([128, 2, 2], F32)
                    nc.vector.tensor_mul(
                        me[:], pse[:, :, b1 - 1 : b1 + 1],
                        ma.rearrange("p (sc b) -> p sc b", sc=2)[:, :, b1 - 1 : b1 + 1],
                    )
                    nc.vector.tensor_add(
                        me[:], me[:],
                        mc.rearrange("p (sc b) -> p sc b", sc=2)[:, :, b1 - 1 : b1 + 1],
                    )
                    nc.scalar.activation(exps[:, :, b1 - 1 : b1 + 1], me[:], AF.Exp)
                else:
                    nc.scalar.activation(
                        exps[:, :, b1 - 1 : b1 + 1], pse[:, :, b1 - 1 : b1 + 1], AF.Exp
                    )
                if pre_ctx is not None:
                    pre_ctx()
                for bb in (b1 - 1, b1):
                    for ec in range(4):
                        for sc in range(2):
                            nc.tensor.matmul(
                                psc[:, ec, bb : bb + 1],
                                ct[:, ((bb * 2 + sc) * 4 + ec) * 128 : ((bb * 2 + sc) * 4 + ec + 1) * 128],
                                exps[:, sc, bb : bb + 1],
                                start=(sc == 0), stop=(sc == 1), skip_group_check=True,
                            )

            for grp in GRPS:
                glen = len(grp)
                tmpg = tmppool.tile([128, glen * 4 * S], BF16, tag="tmpg")
                for j, b in enumerate(grp):
                    for ac in range(4):
                        last_add_inst = nc.vector.tensor_scalar(
                            out=tmpg[:, (j * 4 + ac) * S : (j * 4 + ac + 1) * S],
                            in0=pbuf[:, (b * 4 + ac) * S : (b * 4 + ac + 1) * S],
                            scalar1=tgt_src[:, ac * 8 + b : ac * 8 + b + 1],
                            scalar2=None, op0=ALU.add,
                        )
                thg = thpool.tile([128, glen * 4 * S], BF16, tag="thg")
                if grp[0] == 0:
                    nc.scalar.activation(thg[:, 0 : 2 * S], tmpg[:, 0 : 2 * S], AF.Tanh)
                    nc.scalar.activation(thg[:, 2 * S :], tmpg[:, 2 * S :], AF.Tanh)
                else:
                    nc.scalar.activation(thg[:], tmpg[:], AF.Tanh)
                if debug_t0 and t == 0 and grp[0] == 0:
                    dbgt = smpool.tile([128, 32], F32, tag="dbgt")
                    nc.vector.tensor_copy(dbgt[:], ptgt[:])
                    nc.sync.dma_start(out=d_dbg_tgt, in_=dbgt[:])
                    dbgth = smpool.tile([128, 4 * S], F32, tag="dbgth")
                    nc.vector.tensor_copy(dbgth[:], thg[:, 0 : 4 * S])
                    nc.sync.dma_start(out=d_dbg_th, in_=dbgth[:])
                for j, b in enumerate(grp):
                    for sc in range(2):
                        for ac in range(4):
                            nc.tensor.matmul(
                                pse[:, sc, b : b + 1],
                                thg[:, (j * 4 + ac) * S + sc * 128 : (j * 4 + ac) * S + sc * 128 + 128],
                                va[:, ac : ac + 1],
                                start=(ac == 0), stop=(ac == 3), skip_group_check=True,
                            )
                    if b % 2 == 1:
                        if b < 7:
                            emit_exp_pair(b)

            # last pair: denominator + reciprocal emitted right after the exp,
            # overlapping the final newctx matmuls
            psm = ps_m.tile([128, 16], F32)
            inv = smpool.tile([1, 8], BF16)

            def den_emit():
                nc.tensor.matmul(psm[0:1, 0:8], onesc[:], exps[:, 0, :], start=True, stop=False, skip_group_check=True)
                nc.tensor.matmul(psm[0:1, 0:8], onesc[:], exps[:, 1, :], start=False, stop=True, skip_group_check=True)
                with nc.allow_low_precision(reason="softmax 1/denom in bf16 is enough"):
                    nc.vector.reciprocal(inv[:], psm[0:1, 0:8])

            emit_exp_pair(7, pre_ctx=den_emit)
            nc.tensor.matmul(psm[:, 8:16], onesr[:], inv[:], start=True, stop=True, skip_group_check=True)
            invbc = smpool.tile([128, 1, BL], F32)
            nc.vector.tensor_copy(invbc[:, 0, :], psm[:, 8:16])

            # unnormalized context for the next gx (normalization folded into
            # the gate merge); available right after the newctx matmuls
            cu_new = cxpool.tile([128, 32], BF16, tag="cu")
            nc.vector.tensor_copy(cu_new[:], psc.rearrange("p ec b -> p (ec b)")[:])
            # normalized context for readout/copy-gate (off the critical chain)
            c_new = cxpool.tile([128, 32], BF16)
            c3 = c_new.rearrange("p (ec b) -> p ec b", ec=4)
            with lowprio():
                if bcast:
                    cn_inst = nc.vector.tensor_mul(c3[:], psc[:], invbc.broadcast_to([128, 4, BL]))
                else:
                    for ec in range(4):
                        cn_inst = nc.vector.tensor_mul(c3[:, ec, :], psc[:, ec, :], invbc[:, 0, :])
            with lowprio():
                for sc in range(2):
                    nc.vector.tensor_mul(co_buf[:, sc, t, :], exps[:, sc, :], invbc[:, 0, :])

            if t == T - 1:
                cctxf = smpool.tile([128, 32], F32)
                cc3 = cctxf.rearrange("p (ec b) -> p ec b", ec=4)
                if bcast:
                    nc.vector.tensor_mul(cc3[:], psc[:], invbc.broadcast_to([128, 4, BL]))
                else:
                    for ec in range(4):
                        nc.vector.tensor_mul(cc3[:, ec, :], psc[:, ec, :], invbc[:, 0, :])
                nc.sync.dma_start(out=d_cctx, in_=cctxf[:])

            h_t, c_t = h_new, c_new
            cu_t, inv_cur = cu_new, invbc
            if t + 1 < T:
                psg = psg_next
                if early_gates:
                    # this PSUM->SBUF copy waits on the PE early-gate matmuls;
                    # force it after the attention adds so it cannot stall them
                    a_sb_next = smpool.tile([128, 128], F32, tag="a_sb")
                    with lowprio():
                        cp = nc.vector.tensor_copy(a_sb_next[:], psg_next[:])
                    bass._add_dep_helper(cp.ins, last_add_inst.ins, sync=False,
                                         reason="a_sb copy after attention adds")
                    a_sb_cur = a_sb_next

        # final readout (tau = T-1) and copy-gate activation
        emit_readout(T - 1, h_t, c_t)
        zst = cpool.tile([1, T * BL], F32)
        nc.scalar.activation(zst[:], zs_buf[:], AF.Tanh, scale=0.5, bias=0.5 * b_copy_val)
        zso = cpool.tile([1, T * BL], F32)
        nc.vector.tensor_scalar(
            out=zso[:], in0=zst[:], scalar1=0.5, scalar2=0.5, op0=ALU.mult, op1=ALU.add
        )
        nc.sync.dma_start(out=d_zs, in_=zso[:])
        nc.sync.dma_start(out=d_g, in_=g_buf.rearrange("p a t b -> p (a t b)"))
        nc.sync.dma_start(out=d_co, in_=co_buf.rearrange("p a t b -> p (a t b)"))

    nc.compile()
    return nc


# --------------------------------------------------------------------------
# Host-side packing
# --------------------------------------------------------------------------
def _bf(x):
    return np.ascontiguousarray(x.astype(ml_dtypes.bfloat16))


def _f32(x):
    return np.ascontiguousarray(x.astype(np.float32))


def _prep_inputs(input_ids, hidden, context, src_pad_mask, init_att, emb_table,
                 W_ih, W_hh, b_ih, b_hh, W_pre, b_pre, W_q, v_att,
                 W_copy, b_copy, W_read, b_read):
    nonzero_mask = bool(np.any(src_pad_mask))
    nonzero_bias = bool(
        np.any(b_pre) or np.any(b_hh[1024:]) or np.any(b_read)
    )

    emb = emb_table[input_ids]  # [T, B, WV] gather on host
    bias_fold = b_ih.astype(np.float64).copy()
    bias_fold[:1024] += b_hh[:1024]
    gxe_full = emb.astype(np.float32) @ W_ih[:, :WV].T + bias_fold.astype(np.float32)

    perm = np.concatenate([np.arange(0, DEC, 2), np.arange(1, DEC, 2)])
    wrp = W_read[perm]
    brd_p = b_read[perm]

    def pack_w(wT_src, nm):  # [nm*128, nk*128] -> [128,(kc,mt,m)]
        nk = wT_src.shape[1] // 128
        return _bf(
            wT_src.reshape(nm, 128, nk, 128).transpose(3, 2, 0, 1).reshape(128, -1)
        )

    shared = {
        "wh": pack_w(W_hh, 12),
        "wx": pack_w(W_ih[:, WV:], 12),
        "wq": pack_w(W_q, 4),
        "wr": pack_w(wrp, 4),
        "wp": pack_w(W_pre, 4),
        "wcp": _bf(W_copy.reshape(8, 128).T),
        "va": _bf(v_att.reshape(4, 128).T),
        "ident": _bf(np.eye(128, dtype=np.float32)),
        "onesc": _bf(np.ones((128, 1), np.float32)),
        "onesr": _bf(np.ones((1, 128), np.float32)),
    }
    if nonzero_bias:
        shared["bpre"] = _f32(b_pre.reshape(4, 128).T)
        shared["bhhn"] = _f32(b_hh[1024:].reshape(4, 128).T)
        shared["brd"] = _f32(brd_p.reshape(4, 128).T)

    in_maps = []
    for core in range(NCORES):
        bg = core * BL
        m = dict(shared)
        m["gxe"] = _bf(
            gxe_full[:, bg : bg + BL].reshape(T, BL, 12, 128).transpose(3, 2, 0, 1).reshape(128, -1)
        )
        m["emb"] = _bf(
            emb[:, bg : bg + BL].reshape(T, BL, 4, 128).transpose(3, 2, 0, 1).reshape(128, -1)
        )
        m["ce"] = _bf(
            context[:, bg : bg + BL].reshape(S, BL, 4, 128).transpose(3, 2, 1, 0).reshape(128, -1)
        )
        m["ct"] = _bf(
            context.reshape(2, 128, B, 4, 128)[:, :, bg : bg + BL]
            .transpose(1, 2, 0, 3, 4)
            .reshape(128, -1)
        )
        m["h0"] = _bf(hidden[0, bg : bg + BL].reshape(BL, 4, 128).transpose(2, 1, 0).reshape(128, -1))
        m["c0"] = _bf(init_att[bg : bg + BL].reshape(BL, 4, 128).transpose(2, 1, 0).reshape(128, -1))
        if nonzero_mask:
            msk = src_pad_mask[bg : bg + BL].astype(np.float32)  # [BL, S]
            ma = (1.0 - msk).T.reshape(2, 128, BL).transpose(1, 0, 2).reshape(128, -1)
            mcv = (-1e6 * msk).T.reshape(2, 128, BL).transpose(1, 0, 2).reshape(128, -1)
            m["ma"] = _f32(ma)
            m["mc"] = _f32(mcv)
        in_maps.append(m)
    return in_maps, nonzero_mask, nonzero_bias, float(b_copy[0])


def _unpack(results):
    g_parts, c_parts, z_parts, h_parts, cc_parts = [], [], [], [], []
    for r in results:
        g_parts.append(
            r["g"].reshape(128, 2, T, BL).transpose(2, 3, 1, 0).reshape(T, BL, 256)
        )
        c_parts.append(
            r["co"].astype(np.float32).reshape(128, 2, T, BL).transpose(2, 3, 1, 0).reshape(T, BL, S)
        )
        z_parts.append(r["zs"].reshape(T, BL, 1))
        h_parts.append(r["hid"].reshape(128, 4, BL).transpose(2, 1, 0).reshape(BL, DEC))
        cc_parts.append(r["cctx"].reshape(128, 4, BL).transpose(2, 1, 0).reshape(BL, ENC))
    g_out = np.concatenate(g_parts, axis=1).astype(np.float32)
    c_out = np.concatenate(c_parts, axis=1).astype(np.float32)
    copy_out = np.concatenate(z_parts, axis=1).astype(np.float32)
    hid = np.concatenate(h_parts, axis=0)[None].astype(np.float32)
    cur_ctx = np.concatenate(cc_parts, axis=0).astype(np.float32)
    return g_out, c_out, copy_out, hid, c_out[-1], cur_ctx


def kernel(**inputs):
    inputs = {k: np.asarray(v) for k, v in inputs.items()}
    in_maps, nzm, nzb, b_copy_val = _prep_inputs(**inputs)
    key = (nzm, nzb, b_copy_val)
    if key not in _CACHE:
        _CACHE[key] = _build(nzm, nzb, b_copy_val)
    nc = _CACHE[key]
    res = run_bass_kernel_spmd(nc, in_maps, list(range(NCORES)))
    return _unpack(res.results)
